# revision 51
# baseline (speedup 1.0000x reference)
"""Multi-head causal self-attention on 8 Trainium2 NeuronCores — v2.

Problem: x[2, 2048, 2048], 16 heads x 128 dim, causal softmax, four
2048x2048 projections (nn.Linear convention y = x @ W.T).

Sharding: head tensor-parallel. Core c owns heads {2c, 2c+1}: it computes
those heads' Q/K/V projections, per-head causal attention, and the slice of
the output projection that consumes those heads (Wo columns 256c..256c+256).
Each core emits a full-shape partial output; the host sums the 8 partials.

v5 changes vs v2 (443-459 us baseline -> ~425 us), driven by an NTFF
hardware trace (see _transcript): PE active was 333 us (of which ~160 us
is serial LDWEIGHTS — not modeled by CoreSim), ~98 us PE idle, and the
tail ran 42 us at HAM half-clock:
  - the v2 softmax-denominator chain on GpSimd was the attention-phase
    bottleneck: Pool 2-input bf16 adds measure 1283 ns (3x the cost
    model), making the serial esum chain ~205 us. Moved per-chunk onto
    DVE (474 ns/link): attention-only phase 209 us -> 135 us.
  - per-chunk (512-col) score/exp granularity instead of 1024-col pairs:
    AV waits ~0.5 us for its own chunk's exp, not a whole pair.
  - proj and attention interleaved in program order (attn unit j emitted
    after proj unit j+1) with DEDICATED PSUM pools per use (proj accs 2,
    score chunks 2, O^T accs 2, out-proj 2 banks): sharing one rotation
    serialized the phases through slot allocation.
  - fat DMA: host pre-swizzles x into partition-major block images
    (16KB/partition lines, 1 DMA per half-block) and weights into
    [128, k*dc] images (8KB lines); out stores merged into one
    [128, 2048] stage per 128-token chunk (4KB lines).
  - out-proj emits oc-pairs with hh outer so consecutive matmuls share
    the same stationary ot chunk (repeated LDWEIGHTS are ~76ns cheaper).
v5.2 (trace round 2, ~425 -> ~410 us): tail attention units (emitted
after the last proj block) borrow the idle proj PSUM banks for score
tiles (depth 4 — the exp round-trip stops gating the PE where there is
no proj filler); first wq chunks moved to the ACT queue so they land in
parallel with the first x pieces; xt pool 6 bufs (3 blocks in flight —
the trace showed a recurring 2.9us PE stall per proj block waiting on
its x DMA); stage pool 6.
Measured dead ends kept out: head-ping-pong chunk streams (+35 us),
block-paired weight-stationary projections with V^T+PE-transpose
(+35 us), fp8 (error budget: projections in e4m3 give ~8.6% rms output
error vs the 2e-2 gate).
"""

from contextlib import ExitStack

import numpy as np
import ml_dtypes

import concourse.bacc as bacc
import concourse.mybir as mybir
import concourse.tile as tile
from concourse.bass_utils import run_bass_kernel_spmd

N_CORES = 8
B = 2
SEQ = 2048
H = 2048
NHEADS = 16
D = 128
HPC = NHEADS // N_CORES  # heads per core
DC = HPC * D             # per-core head dims (256)
QB = 512                 # q/token block (moving free dim)
KTH = H // 128           # 16 contraction tiles over hidden
SCALE = 1.0 / float(np.sqrt(D))

F32 = mybir.dt.float32
BF16 = mybir.dt.bfloat16

EXP = mybir.ActivationFunctionType.Exp


def build(seq=SEQ, reps=1, only=None, bufs=None):
    """Emit the per-core program. seq is parameterized for small dev runs."""
    t = B * seq
    nblocks = seq // QB          # token blocks per batch
    nchunks = seq // 128         # 128-token chunks per batch

    nc = bacc.Bacc("TRN2", target_bir_lowering=False, debug=False,
                   num_devices=N_CORES)
    # Host pre-swizzles everything into partition-major images so every DMA
    # has fat (>=2KB) per-partition lines:
    #   xb [blk*128+p, k*QB+t] = x^T[k*128+p, blk*QB+t]   (16KB lines)
    #   w{q,k,v}b [p, k*DC+c] = W_c.T[k*128+p, c]          (8KB lines)
    #   wot [DC, H] as before                              (4KB lines)
    xb_ap = nc.dram_tensor("xb_in", [(t // QB) * 128, KTH * QB], BF16,
                           kind="ExternalInput").ap()
    wqt_ap = nc.dram_tensor("wqb", [128, KTH * DC], BF16,
                            kind="ExternalInput").ap()
    wkt_ap = nc.dram_tensor("wkb", [128, KTH * DC], BF16,
                            kind="ExternalInput").ap()
    wvt_ap = nc.dram_tensor("wvb", [128, KTH * DC], BF16,
                            kind="ExternalInput").ap()
    wot_ap = nc.dram_tensor("wot", [DC, H], BF16, kind="ExternalInput").ap()
    out_ap = nc.dram_tensor("out", [t, H], BF16, kind="ExternalOutput").ap()

    with tile.TileContext(nc) as tc, ExitStack() as ctx:
        const = ctx.enter_context(tc.tile_pool(name="const", bufs=1))
        # ones MATRIX: z matmul with lhsT=ones[128,128] writes the partition
        # sum to EVERY output partition — denominator + broadcast in one
        # 213ns matmul.
        ones_f32 = const.tile([128, 128], F32, name="ones_f32")
        nc.gpsimd.memset(ones_f32[:], 1.0)
        ones = const.tile([128, 128], BF16, name="ones")
        nc.vector.tensor_copy(ones[:], ones_f32[:])
        # Multiplicative causal mask: maskw[p, w] = 1 if (w - p - 384) >= 0
        # else 0. Diagonal chunk with m = kc-(n_kc-4) uses cols
        # [384-128m, +512); applied to E^T after exp.
        maskw_f32 = const.tile([128, 896], F32, name="maskw_f32")
        nc.gpsimd.memset(maskw_f32[:], 1.0)
        nc.gpsimd.affine_select(
            out=maskw_f32[:], in_=maskw_f32[:],
            compare_op=mybir.AluOpType.is_ge,
            fill=0.0, base=-384,
            pattern=[[1, 896]], channel_multiplier=-1,
        )
        maskw = const.tile([128, 896], BF16, name="maskw")
        nc.vector.tensor_copy(maskw[:], maskw_f32[:])
        # identity for PE-mode transpose (V^T -> V)
        ident_f32 = const.tile([128, 128], F32, name="ident_f32")
        nc.gpsimd.memset(ident_f32[:], 1.0)
        nc.gpsimd.affine_select(
            out=ident_f32[:], in_=ident_f32[:],
            compare_op=mybir.AluOpType.is_equal,
            fill=0.0, base=0,
            pattern=[[1, 128]], channel_multiplier=-1,
        )
        ident = const.tile([128, 128], BF16, name="ident")
        nc.vector.tensor_copy(ident[:], ident_f32[:])

        # --- resident weights ---
        wt_pool = ctx.enter_context(tc.tile_pool(name="wt", bufs=1))
        wqkvT = {
            nm: wt_pool.tile([128, KTH * DC], BF16, name=f"w{nm}T", tag=f"w{nm}T")
            for nm in ("q", "k", "v")
        }
        woT = wt_pool.tile([128, HPC * H], BF16, name="woT", tag="woT")

        bd = {"et": 12, "xt": 8, "otb": 2, "stage": 6, "esum": 2,
              "look": 4, "lag": 1,
              "projps": 2, "stps": 2, "otps": 2, "opps": 2}
        bd.update(bufs or {})
        # PSUM: 8 banks split into four DEDICATED pools. Sharing one
        # rotation between proj accumulators and attention's long-lived
        # ot_ps/op_ps tenants serializes the phases through slot allocation
        # (proj matmuls end up waiting on the previous attention block's
        # out-projection) — dedicated pools let them truly interleave.
        projps = ctx.enter_context(tc.tile_pool(name="projps",
                                                bufs=bd["projps"],
                                                space="PSUM"))
        stpool = ctx.enter_context(tc.tile_pool(name="stps", bufs=bd["stps"],
                                                space="PSUM"))
        otps_pool = ctx.enter_context(tc.tile_pool(name="otps",
                                                   bufs=bd["otps"],
                                                   space="PSUM"))
        oppool = ctx.enter_context(tc.tile_pool(name="opps", bufs=bd["opps"],
                                                space="PSUM"))

        # Weight loads: the first wq chunks ride the ACT queue (empty at
        # startup) so they land in PARALLEL with the first x pieces on the
        # SP queue instead of serializing in front of them.
        nc.scalar.dma_start(wqkvT["q"][:, 0:4 * DC], wqt_ap[:, 0:4 * DC])
        nc.scalar.dma_start(wqkvT["q"][:, 4 * DC:], wqt_ap[:, 4 * DC:])
        nc.scalar.dma_start(wqkvT["k"][:], wkt_ap)
        nc.scalar.dma_start(wqkvT["v"][:], wvt_ap)
        nc.scalar.dma_start(
            woT[:].rearrange("p (hh o) -> p hh o", o=H),
            wot_ap.rearrange("(hh p) o -> p hh o", p=128))

        qkv_pool = ctx.enter_context(tc.tile_pool(name="qkv", bufs=2))
        xt_pool = ctx.enter_context(tc.tile_pool(name="xt", bufs=bd["xt"]))
        et_pool = ctx.enter_context(tc.tile_pool(name="et", bufs=bd["et"]))
        ot_pool = ctx.enter_context(tc.tile_pool(name="otb", bufs=bd["otb"]))
        stage_pool = ctx.enter_context(
            tc.tile_pool(name="stage", bufs=bd["stage"]))
        esum_pool = ctx.enter_context(
            tc.tile_pool(name="esum", bufs=bd["esum"]))
        look = bd["look"]

        def body():
            tiles = {}

            def ensure_tiles(b):
                if b in tiles:
                    return tiles[b]
                qt_sb = [qkv_pool.tile([128, seq], BF16, tag=f"qt{h}",
                                       name=f"qt{h}") for h in range(HPC)]
                kt_sb = [qkv_pool.tile([128, seq], BF16, tag=f"kt{h}",
                                       name=f"kt{h}") for h in range(HPC)]
                vn_sb = qkv_pool.tile([128, nchunks * DC], BF16, tag="vn",
                                      name="vn")
                if (only or "").startswith('attn'):
                    for tb in qt_sb + kt_sb + [vn_sb]:
                        nc.vector.memset(tb[:], 0.0)
                tiles[b] = (qt_sb, kt_sb, vn_sb, None)
                return tiles[b]

            if True:
                # phase 1: Q^T/K^T [d, tok], V [tok, d] for one block.
                def proj_block(b, nb):
                    qt_sb, kt_sb, vn_sb, vt_sb = ensure_tiles(b)
                    blk = b * nblocks + nb
                    row0 = blk * 128
                    halves = [xt_pool.tile([128, KTH // 2 * QB], BF16,
                                           tag="xt", name="xt")
                              for _ in range(2)]
                    xts = [halves[kt // (KTH // 2)]
                           [:, (kt % (KTH // 2)) * QB:
                            (kt % (KTH // 2) + 1) * QB]
                           for kt in range(KTH)]
                    # First block of the run: fine-grained pieces so the
                    # first Q matmuls start as soon as kt chunk 0 lands.
                    # Later blocks: one fat 1MB DMA per half (16KB lines).
                    kpp = 2 if (b == 0 and nb == 0) else 8
                    for hf in range(2):
                        for hq in range(8 // kpp):
                            c0 = (hf * 8 + hq * kpp) * QB
                            c1 = c0 + kpp * QB
                            nc.sync.dma_start(
                                halves[hf][:, hq * kpp * QB:
                                           (hq + 1) * kpp * QB],
                                xb_ap[row0:row0 + 128, c0:c1])
                    for nm_p, dst in (("q", qt_sb), ("k", kt_sb)):
                        acc = [projps.tile([128, QB], F32, tag="projps",
                                           name="pacc") for _ in range(HPC)]
                        for kt in range(KTH):
                            first, last = kt == 0, kt == KTH - 1
                            for hh in range(HPC):
                                col = kt * DC + hh * 128
                                nc.tensor.matmul(
                                    acc[hh][:],
                                    (wqkvT[nm_p][:, col:col + 128]),
                                    (xts[kt][:]), start=first, stop=last)
                        for hh in range(HPC):
                            nc.scalar.copy(
                                dst[hh][:, nb * QB:(nb + 1) * QB], acc[hh][:])
                    for c4 in range(QB // 128):
                        vn_ps = projps.tile([128, DC], F32, tag="projps",
                                            name="vnps")
                        for kt in range(KTH):
                            nc.tensor.matmul(
                                vn_ps[:],
                                (xts[kt][:, c4 * 128:(c4 + 1) * 128]),
                                (wqkvT["v"][:, kt * DC:(kt + 1) * DC]),
                                start=(kt == 0), stop=(kt == KTH - 1))
                        chunk = nb * (QB // 128) + c4
                        nc.scalar.copy(
                            vn_sb[:, chunk * DC:(chunk + 1) * DC], vn_ps[:])

                # phase 2+3: attention + output projection for one q block
                def attn_block(b, qb, mode="full", deep=False):
                    qt_sb, kt_sb, vn_sb, vt_sb = ensure_tiles(b)
                    q0 = qb * QB
                    n_kc = (qb + 1) * (QB // 128)
                    ot_sbs = []
                    for hh in range(HPC):
                        esum = esum_pool.tile([128, QB], BF16,
                                              tag=f"esum{hh}", name="esum")

                        def score_chunk(kc):
                            # Diagonal chunks: cols < 128m are entirely
                            # non-causal — skip them in the score matmul,
                            # exp, and AV. The triangle window itself is a
                            # mask multiply; skipped cols never see exp.
                            m = kc - (n_kc - 4)
                            w0 = 128 * m if m > 0 else 0
                            # tail units (no proj filler behind them) borrow
                            # the idle proj PSUM banks: score depth 4, so
                            # the exp round-trip stops gating the PE there
                            sp = projps if (deep and kc % 2) else stpool
                            st_ps = sp.tile([128, QB], F32,
                                            tag="projps" if sp is projps
                                            else "stps", name="stps")
                            et = et_pool.tile([128, QB], BF16, tag="et",
                                              name="et")
                            nc.tensor.matmul(
                                st_ps[:, w0:QB],
                                (kt_sb[hh][:, kc * 128:(kc + 1) * 128]),
                                (qt_sb[hh][:, q0 + w0:q0 + QB]),
                                start=True, stop=True)
                            if mode == "peonly":
                                # timing probe: PE stream + PSUM evictions
                                # only — exp replaced by a plain ACT copy,
                                # no mask/esum (et feeds AV directly)
                                nc.scalar.copy(et[:, w0:QB], st_ps[:, w0:QB])
                                return et
                            nc.scalar.activation(et[:, w0:QB],
                                                 st_ps[:, w0:QB],
                                                 EXP, scale=SCALE)
                            if m >= 0:
                                if w0 > 0:
                                    nc.gpsimd.memset(et[:, 0:w0], 0.0)
                                # triangle window: keep col j iff j >= p
                                sl = et[:, w0:w0 + 128]
                                nc.vector.tensor_mul(sl, sl,
                                                     maskw[:, 384:512])
                            # E_sum chain per chunk on DVE (bf16 SBUF):
                            # ~474ns/link vs Pool's 1283ns — the Pool chain
                            # was the whole-phase bottleneck in v2.
                            if kc == 0:
                                nc.vector.tensor_copy(esum[:], et[:])
                            else:
                                nc.vector.tensor_add(esum[:], esum[:], et[:])
                            return et

                        ot_ps = otps_pool.tile([128, QB], F32, tag="otps",
                                               name="otps")
                        ot_sb = ot_pool.tile([128, QB], BF16, tag=f"ot{hh}",
                                             name=f"ot{hh}")
                        zbr = ot_pool.tile([128, QB], F32, tag="zbr",
                                           name="zbr")
                        ets = {kc: score_chunk(kc)
                               for kc in range(min(look, n_kc))}
                        for kc in range(n_kc):
                            if kc + look < n_kc:
                                ets[kc + look] = score_chunk(kc + look)
                            et = ets.pop(kc)
                            col = kc * DC + hh * 128
                            # Diagonal chunks contribute only to cols >=
                            # 128m (et is zero left of that) — accumulate
                            # the sub-range only. Col 0 gets start=True
                            # from the full-width kc=0 chunk, and each
                            # diagonal chunk is the LAST writer of its
                            # leftmost 128-col window, so hardware
                            # accumulate semantics are unchanged.
                            m = kc - (n_kc - 4)
                            w0 = 128 * m if m > 0 else 0
                            nc.tensor.matmul(
                                ot_ps[:, w0:QB], (vn_sb[:, col:col + 128]),
                                (et[:, w0:QB]),
                                start=(kc == 0), stop=(kc == n_kc - 1),
                                skip_group_check=(m > 0))
                            if m == 1 and mode != "peonly":
                                # z a few AV matmuls early: ones[128,128]
                                # writes the denominator row to every PSUM
                                # partition (broadcast free), and the
                                # reciprocal hides under remaining AVs.
                                # zb borrows a score slot: no score allocs
                                # remain in this unit after m==1 fires.
                                zb_ps = stpool.tile([128, QB], F32,
                                                    tag="stps", name="zbps")
                                nc.tensor.matmul(zb_ps[:], (ones[:]),
                                                 (esum[:]),
                                                 start=True, stop=True)
                                nc.vector.reciprocal(zbr[:], zb_ps[:])
                        if mode == "peonly":
                            nc.scalar.copy(ot_sb[:], ot_ps[:])
                        else:
                            nc.vector.tensor_mul(ot_sb[:], ot_ps[:], zbr[:])
                        ot_sbs.append(ot_sb)
                    if mode == "noop":
                        return
                    for c4 in range(QB // 128):
                        row0 = b * seq + q0 + c4 * 128
                        # one [128, H] stage per 128-token chunk: 4 PSUM
                        # evictions (split ACT/DVE) then a single fat store
                        # (4KB DRAM lines)
                        stg = stage_pool.tile([128, H], BF16,
                                              tag="stage", name="stg")
                        last = b == B - 1 and qb == nblocks - 1
                        # oc pairs with hh outer: consecutive matmuls share
                        # the same stationary ot chunk — repeated LDWEIGHTS
                        # of identical weights measure ~76ns cheaper.
                        for ocp in range(H // QB // 2):
                            ops = [oppool.tile([128, QB], F32, tag="opps",
                                               name="opps")
                                   for _ in range(2)]
                            for hh in range(HPC):
                                for j in range(2):
                                    oc = 2 * ocp + j
                                    nc.tensor.matmul(
                                        ops[j][:],
                                        (ot_sbs[hh][:,
                                                    c4 * 128:(c4 + 1) * 128]),
                                        (woT[:, hh * H + oc * QB:
                                               hh * H + (oc + 1) * QB]),
                                        start=(hh == 0),
                                        stop=(hh == HPC - 1))
                            for j in range(2):
                                oc = 2 * ocp + j
                                sl = stg[:, oc * QB:(oc + 1) * QB]
                                if j == 1:
                                    # DVE carries the esum chain; split the
                                    # PSUM evictions evenly with ACT
                                    nc.scalar.copy(sl, ops[j][:])
                                else:
                                    nc.vector.tensor_copy(sl, ops[j][:])
                        if mode == "nostore":
                            continue
                        # stores ride the ACT queue (SP carries x loads);
                        # final block splits across both for the tail drain
                        eng = nc.sync if (last and c4 % 2) else nc.scalar
                        eng.dma_start(out_ap[row0:row0 + 128, :], stg[:])

                amode = ("full" if only in (None, 'attn') else
                         (only or "").removeprefix('attn_'))
                lag = bd["lag"]
                nP = B * nblocks
                if only == 'proj':
                    order = [("P", i) for i in range(nP)]
                elif (only or "").startswith('attn'):
                    order = [("A", i) for i in range(nP)]
                else:
                    # interleave: attn unit j is emitted after proj unit
                    # j+lag, so attention stalls always have dense proj
                    # matmuls available behind them in the engine streams.
                    order = []
                    for i in range(nP):
                        order.append(("P", i))
                        if i >= lag:
                            order.append(("A", i - lag))
                    for j in range(nP - lag, nP):
                        order.append(("A", j))
                last_p = max((k for k, (kd, _) in enumerate(order)
                              if kd == "P"), default=-1)
                for k, (kind, i) in enumerate(order):
                    if kind == "P":
                        proj_block(i // nblocks, i % nblocks)
                    else:
                        attn_block(i // nblocks, i % nblocks, amode,
                                   deep=(k > last_p))

        if reps == 1:
            body()
        else:
            with tc.For_i(0, reps, 1):
                body()

    nc.compile()
    return nc


def shard_inputs(x, Wq, Wk, Wv, Wo, seq=SEQ):
    t = B * seq
    bf = ml_dtypes.bfloat16
    x2t = np.asarray(x, dtype=np.float32).reshape(t, H).T  # [H, t]
    # xb[blk*128+p, k*QB+tt] = x^T[k*128+p, blk*QB+tt] — partition-major
    # blocks so each per-partition DMA line is 16KB contiguous.
    xb = np.ascontiguousarray(
        x2t.reshape(KTH, 128, t // QB, QB).transpose(2, 1, 0, 3)
        .reshape((t // QB) * 128, KTH * QB)).astype(bf)

    def wswz(w):  # [H, DC] -> [128, KTH*DC] partition-major
        return np.ascontiguousarray(
            w.reshape(KTH, 128, DC).transpose(1, 0, 2).reshape(128, KTH * DC)
        ).astype(bf)

    Wq = np.asarray(Wq, dtype=np.float32)
    Wk = np.asarray(Wk, dtype=np.float32)
    Wv = np.asarray(Wv, dtype=np.float32)
    Wo = np.asarray(Wo, dtype=np.float32)
    in_maps = []
    for c in range(N_CORES):
        sl = slice(c * DC, (c + 1) * DC)
        in_maps.append({
            "xb_in": xb,
            "wqb": wswz(np.ascontiguousarray(Wq[sl, :].T)),
            "wkb": wswz(np.ascontiguousarray(Wk[sl, :].T)),
            "wvb": wswz(np.ascontiguousarray(Wv[sl, :].T)),
            "wot": np.ascontiguousarray(Wo[:, sl].T).astype(bf),
        })
    return in_maps


_cache = {}


def kernel(x, Wq, Wk, Wv, Wo):
    if "nc" not in _cache:
        _cache["nc"] = build()
    nc = _cache["nc"]
    in_maps = shard_inputs(x, Wq, Wk, Wv, Wo)
    res = run_bass_kernel_spmd(nc, in_maps, list(range(N_CORES)))
    acc = res.results[0]["out"].astype(np.float32)
    for c in range(1, N_CORES):
        acc = acc + res.results[c]["out"].astype(np.float32)
    return acc.reshape(B, SEQ, H)



# revision 53
# speedup vs baseline: 1.0694x; 1.0694x over previous
"""Multi-head causal self-attention on 8 Trainium2 NeuronCores — v2.

Problem: x[2, 2048, 2048], 16 heads x 128 dim, causal softmax, four
2048x2048 projections (nn.Linear convention y = x @ W.T).

Sharding: head tensor-parallel. Core c owns heads {2c, 2c+1}: it computes
those heads' Q/K/V projections, per-head causal attention, and the slice of
the output projection that consumes those heads (Wo columns 256c..256c+256).
Each core emits a full-shape partial output; the host sums the 8 partials.

v5 changes vs v2 (443-459 us baseline -> ~425 us), driven by an NTFF
hardware trace (see _transcript): PE active was 333 us (of which ~160 us
is serial LDWEIGHTS — not modeled by CoreSim), ~98 us PE idle, and the
tail ran 42 us at HAM half-clock:
  - the v2 softmax-denominator chain on GpSimd was the attention-phase
    bottleneck: Pool 2-input bf16 adds measure 1283 ns (3x the cost
    model), making the serial esum chain ~205 us. Moved per-chunk onto
    DVE (474 ns/link): attention-only phase 209 us -> 135 us.
  - per-chunk (512-col) score/exp granularity instead of 1024-col pairs:
    AV waits ~0.5 us for its own chunk's exp, not a whole pair.
  - proj and attention interleaved in program order (attn unit j emitted
    after proj unit j+1) with DEDICATED PSUM pools per use (proj accs 2,
    score chunks 2, O^T accs 2, out-proj 2 banks): sharing one rotation
    serialized the phases through slot allocation.
  - fat DMA: host pre-swizzles x into partition-major block images
    (16KB/partition lines, 1 DMA per half-block) and weights into
    [128, k*dc] images (8KB lines); out stores merged into one
    [128, 2048] stage per 128-token chunk (4KB lines).
  - out-proj emits oc-pairs with hh outer so consecutive matmuls share
    the same stationary ot chunk (repeated LDWEIGHTS are ~76ns cheaper).
v5.2 (trace round 2, ~425 -> ~410 us): tail attention units (emitted
after the last proj block) borrow the idle proj PSUM banks for score
tiles (depth 4 — the exp round-trip stops gating the PE where there is
no proj filler); first wq chunks moved to the ACT queue so they land in
parallel with the first x pieces; xt pool 6 bufs (3 blocks in flight —
the trace showed a recurring 2.9us PE stall per proj block waiting on
its x DMA); stage pool 6.
Measured dead ends kept out: head-ping-pong chunk streams (+35 us),
block-paired weight-stationary projections with V^T+PE-transpose
(+35 us), fp8 (error budget: projections in e4m3 give ~8.6% rms output
error vs the 2e-2 gate).
"""

from contextlib import ExitStack

import numpy as np
import ml_dtypes

import concourse.bacc as bacc
import concourse.mybir as mybir
import concourse.tile as tile
from concourse.bass_utils import run_bass_kernel_spmd

N_CORES = 8
B = 2
SEQ = 2048
H = 2048
NHEADS = 16
D = 128
HPC = NHEADS // N_CORES  # heads per core
DC = HPC * D             # per-core head dims (256)
QB = 512                 # q/token block (moving free dim)
KTH = H // 128           # 16 contraction tiles over hidden
SCALE = 1.0 / float(np.sqrt(D))

F32 = mybir.dt.float32
BF16 = mybir.dt.bfloat16

EXP = mybir.ActivationFunctionType.Exp


def build(seq=SEQ, reps=1, only=None, bufs=None):
    """Emit the per-core program. seq is parameterized for small dev runs."""
    t = B * seq
    nblocks = seq // QB          # token blocks per batch
    nchunks = seq // 128         # 128-token chunks per batch

    nc = bacc.Bacc("TRN2", target_bir_lowering=False, debug=False,
                   num_devices=N_CORES)
    # Host pre-swizzles everything into partition-major images so every DMA
    # has fat (>=2KB) per-partition lines:
    #   xb [blk*128+p, k*QB+t] = x^T[k*128+p, blk*QB+t]   (16KB lines)
    #   w{q,k,v}b [p, k*DC+c] = W_c.T[k*128+p, c]          (8KB lines)
    #   wot [DC, H] as before                              (4KB lines)
    xb_ap = nc.dram_tensor("xb_in", [(t // QB) * 128, KTH * QB], BF16,
                           kind="ExternalInput").ap()
    wqt_ap = nc.dram_tensor("wqb", [128, KTH * DC], BF16,
                            kind="ExternalInput").ap()
    wkt_ap = nc.dram_tensor("wkb", [128, KTH * DC], BF16,
                            kind="ExternalInput").ap()
    wvt_ap = nc.dram_tensor("wvb", [128, KTH * DC], BF16,
                            kind="ExternalInput").ap()
    wot_ap = nc.dram_tensor("wot", [DC, H], BF16, kind="ExternalInput").ap()
    out_ap = nc.dram_tensor("out", [t, H], BF16, kind="ExternalOutput").ap()

    with tile.TileContext(nc) as tc, ExitStack() as ctx:
        const = ctx.enter_context(tc.tile_pool(name="const", bufs=1))
        # ones MATRIX: z matmul with lhsT=ones[128,128] writes the partition
        # sum to EVERY output partition — denominator + broadcast in one
        # 213ns matmul.
        ones_f32 = const.tile([128, 128], F32, name="ones_f32")
        nc.gpsimd.memset(ones_f32[:], 1.0)
        ones = const.tile([128, 128], BF16, name="ones")
        nc.vector.tensor_copy(ones[:], ones_f32[:])
        # Multiplicative causal mask: maskw[p, w] = 1 if (w - p - 384) >= 0
        # else 0. Diagonal chunk with m = kc-(n_kc-4) uses cols
        # [384-128m, +512); applied to E^T after exp.
        maskw_f32 = const.tile([128, 896], F32, name="maskw_f32")
        nc.gpsimd.memset(maskw_f32[:], 1.0)
        nc.gpsimd.affine_select(
            out=maskw_f32[:], in_=maskw_f32[:],
            compare_op=mybir.AluOpType.is_ge,
            fill=0.0, base=-384,
            pattern=[[1, 896]], channel_multiplier=-1,
        )
        maskw = const.tile([128, 896], BF16, name="maskw")
        nc.vector.tensor_copy(maskw[:], maskw_f32[:])
        # identity for PE-mode transpose (V^T -> V)
        ident_f32 = const.tile([128, 128], F32, name="ident_f32")
        nc.gpsimd.memset(ident_f32[:], 1.0)
        nc.gpsimd.affine_select(
            out=ident_f32[:], in_=ident_f32[:],
            compare_op=mybir.AluOpType.is_equal,
            fill=0.0, base=0,
            pattern=[[1, 128]], channel_multiplier=-1,
        )
        ident = const.tile([128, 128], BF16, name="ident")
        nc.vector.tensor_copy(ident[:], ident_f32[:])

        # --- resident weights ---
        wt_pool = ctx.enter_context(tc.tile_pool(name="wt", bufs=1))
        wqkvT = {
            nm: wt_pool.tile([128, KTH * DC], BF16, name=f"w{nm}T", tag=f"w{nm}T")
            for nm in ("q", "k", "v")
        }
        woT = wt_pool.tile([128, HPC * H], BF16, name="woT", tag="woT")

        bd = {"et": 8, "xt": 6, "otb": 2, "stage": 6, "esum": 2,
              "look": 4, "lag": 1,
              "projps": 2, "stps": 2, "otps": 2, "opps": 2}
        bd.update(bufs or {})
        # PSUM: 8 banks split into four DEDICATED pools. Sharing one
        # rotation between proj accumulators and attention's long-lived
        # ot_ps/op_ps tenants serializes the phases through slot allocation
        # (proj matmuls end up waiting on the previous attention block's
        # out-projection) — dedicated pools let them truly interleave.
        projps = ctx.enter_context(tc.tile_pool(name="projps",
                                                bufs=bd["projps"],
                                                space="PSUM"))
        stpool = ctx.enter_context(tc.tile_pool(name="stps", bufs=bd["stps"],
                                                space="PSUM"))
        otps_pool = ctx.enter_context(tc.tile_pool(name="otps",
                                                   bufs=bd["otps"],
                                                   space="PSUM"))
        oppool = ctx.enter_context(tc.tile_pool(name="opps", bufs=bd["opps"],
                                                space="PSUM"))

        # Weight loads: the first wq chunks ride the ACT queue (empty at
        # startup) so they land in PARALLEL with the first x pieces on the
        # SP queue instead of serializing in front of them.
        nc.scalar.dma_start(wqkvT["q"][:, 0:4 * DC], wqt_ap[:, 0:4 * DC])
        nc.scalar.dma_start(wqkvT["q"][:, 4 * DC:], wqt_ap[:, 4 * DC:])
        nc.scalar.dma_start(wqkvT["k"][:], wkt_ap)
        nc.scalar.dma_start(wqkvT["v"][:], wvt_ap)
        nc.scalar.dma_start(
            woT[:].rearrange("p (hh o) -> p hh o", o=H),
            wot_ap.rearrange("(hh p) o -> p hh o", p=128))

        qkv_pool = ctx.enter_context(tc.tile_pool(name="qkv", bufs=2))
        xt_pool = ctx.enter_context(tc.tile_pool(name="xt", bufs=bd["xt"]))
        et_pool = ctx.enter_context(tc.tile_pool(name="et", bufs=bd["et"]))
        ot_pool = ctx.enter_context(tc.tile_pool(name="otb", bufs=bd["otb"]))
        stage_pool = ctx.enter_context(
            tc.tile_pool(name="stage", bufs=bd["stage"]))
        esum_pool = ctx.enter_context(
            tc.tile_pool(name="esum", bufs=bd["esum"]))
        look = bd["look"]

        def body():
            tiles = {}

            def ensure_tiles(b):
                if b in tiles:
                    return tiles[b]
                qt_sb = [qkv_pool.tile([128, seq], BF16, tag=f"qt{h}",
                                       name=f"qt{h}") for h in range(HPC)]
                kt_sb = [qkv_pool.tile([128, seq], BF16, tag=f"kt{h}",
                                       name=f"kt{h}") for h in range(HPC)]
                vn_sb = qkv_pool.tile([128, nchunks * DC], BF16, tag="vn",
                                      name="vn")
                if (only or "").startswith('attn'):
                    for tb in qt_sb + kt_sb + [vn_sb]:
                        nc.vector.memset(tb[:], 0.0)
                tiles[b] = (qt_sb, kt_sb, vn_sb, None)
                return tiles[b]

            if True:
                # phase 1: Q^T/K^T [d, tok], V [tok, d] for one block.
                def proj_block(b, nb):
                    qt_sb, kt_sb, vn_sb, vt_sb = ensure_tiles(b)
                    blk = b * nblocks + nb
                    row0 = blk * 128
                    halves = [xt_pool.tile([128, KTH // 2 * QB], BF16,
                                           tag="xt", name="xt")
                              for _ in range(2)]
                    xts = [halves[kt // (KTH // 2)]
                           [:, (kt % (KTH // 2)) * QB:
                            (kt % (KTH // 2) + 1) * QB]
                           for kt in range(KTH)]
                    # First block of the run: fine-grained pieces so the
                    # first Q matmuls start as soon as kt chunk 0 lands.
                    # Later blocks: one fat 1MB DMA per half (16KB lines).
                    kpp = 2 if (b == 0 and nb == 0) else 8
                    for hf in range(2):
                        for hq in range(8 // kpp):
                            c0 = (hf * 8 + hq * kpp) * QB
                            c1 = c0 + kpp * QB
                            # steady-state block loads ride the SWDGE
                            # (gpsimd) queue: the SP HWDGE ring is strict
                            # FIFO, so a WAR-gated load at its head delayed
                            # every following load (~2.9us PE stall per
                            # block in the trace). First block stays on the
                            # lower-latency HWDGE path.
                            eng = nc.sync if kpp == 2 else nc.gpsimd
                            eng.dma_start(
                                halves[hf][:, hq * kpp * QB:
                                           (hq + 1) * kpp * QB],
                                xb_ap[row0:row0 + 128, c0:c1])
                    for nm_p, dst in (("q", qt_sb), ("k", kt_sb)):
                        acc = [projps.tile([128, QB], F32, tag="projps",
                                           name="pacc") for _ in range(HPC)]
                        for kt in range(KTH):
                            first, last = kt == 0, kt == KTH - 1
                            for hh in range(HPC):
                                col = kt * DC + hh * 128
                                nc.tensor.matmul(
                                    acc[hh][:],
                                    (wqkvT[nm_p][:, col:col + 128]),
                                    (xts[kt][:]), start=first, stop=last)
                        for hh in range(HPC):
                            nc.scalar.copy(
                                dst[hh][:, nb * QB:(nb + 1) * QB], acc[hh][:])
                    for c4 in range(QB // 128):
                        vn_ps = projps.tile([128, DC], F32, tag="projps",
                                            name="vnps")
                        for kt in range(KTH):
                            nc.tensor.matmul(
                                vn_ps[:],
                                (xts[kt][:, c4 * 128:(c4 + 1) * 128]),
                                (wqkvT["v"][:, kt * DC:(kt + 1) * DC]),
                                start=(kt == 0), stop=(kt == KTH - 1))
                        chunk = nb * (QB // 128) + c4
                        nc.scalar.copy(
                            vn_sb[:, chunk * DC:(chunk + 1) * DC], vn_ps[:])

                # phase 2+3: attention + output projection for one q block
                def attn_block(b, qb, mode="full", deep=False):
                    qt_sb, kt_sb, vn_sb, vt_sb = ensure_tiles(b)
                    q0 = qb * QB
                    n_kc = (qb + 1) * (QB // 128)
                    ot_sbs = []
                    for hh in range(HPC):
                        esum = esum_pool.tile([128, QB], BF16,
                                              tag=f"esum{hh}", name="esum")

                        def score_chunk(kc):
                            # Diagonal chunks: cols < 128m are entirely
                            # non-causal — skip them in the score matmul,
                            # exp, and AV. The triangle window itself is a
                            # mask multiply; skipped cols never see exp.
                            m = kc - (n_kc - 4)
                            w0 = 128 * m if m > 0 else 0
                            # tail units (no proj filler behind them) borrow
                            # the idle proj PSUM banks: score depth 4, so
                            # the exp round-trip stops gating the PE there
                            sp = projps if (deep and kc % 2) else stpool
                            st_ps = sp.tile([128, QB], F32,
                                            tag="projps" if sp is projps
                                            else "stps", name="stps")
                            et = et_pool.tile([128, QB], BF16, tag="et",
                                              name="et")
                            nc.tensor.matmul(
                                st_ps[:, w0:QB],
                                (kt_sb[hh][:, kc * 128:(kc + 1) * 128]),
                                (qt_sb[hh][:, q0 + w0:q0 + QB]),
                                start=True, stop=True)
                            if mode == "peonly":
                                # timing probe: PE stream + PSUM evictions
                                # only — exp replaced by a plain ACT copy,
                                # no mask/esum (et feeds AV directly)
                                nc.scalar.copy(et[:, w0:QB], st_ps[:, w0:QB])
                                return et
                            nc.scalar.activation(et[:, w0:QB],
                                                 st_ps[:, w0:QB],
                                                 EXP, scale=SCALE)
                            if m >= 0:
                                if w0 > 0:
                                    nc.gpsimd.memset(et[:, 0:w0], 0.0)
                                # triangle window: keep col j iff j >= p
                                sl = et[:, w0:w0 + 128]
                                nc.vector.tensor_mul(sl, sl,
                                                     maskw[:, 384:512])
                            # E_sum chain per chunk on DVE (bf16 SBUF):
                            # ~474ns/link vs Pool's 1283ns — the Pool chain
                            # was the whole-phase bottleneck in v2.
                            if kc == 0:
                                nc.vector.tensor_copy(esum[:], et[:])
                            else:
                                nc.vector.tensor_add(esum[:], esum[:], et[:])
                            return et

                        ot_ps = otps_pool.tile([128, QB], F32, tag="otps",
                                               name="otps")
                        ot_sb = ot_pool.tile([128, QB], BF16, tag=f"ot{hh}",
                                             name=f"ot{hh}")
                        zbr = ot_pool.tile([128, QB], F32, tag="zbr",
                                           name="zbr")
                        ets = {kc: score_chunk(kc)
                               for kc in range(min(look, n_kc))}
                        for kc in range(n_kc):
                            if kc + look < n_kc:
                                ets[kc + look] = score_chunk(kc + look)
                            et = ets.pop(kc)
                            col = kc * DC + hh * 128
                            # Diagonal chunks contribute only to cols >=
                            # 128m (et is zero left of that) — accumulate
                            # the sub-range only. Col 0 gets start=True
                            # from the full-width kc=0 chunk, and each
                            # diagonal chunk is the LAST writer of its
                            # leftmost 128-col window, so hardware
                            # accumulate semantics are unchanged.
                            m = kc - (n_kc - 4)
                            w0 = 128 * m if m > 0 else 0
                            nc.tensor.matmul(
                                ot_ps[:, w0:QB], (vn_sb[:, col:col + 128]),
                                (et[:, w0:QB]),
                                start=(kc == 0), stop=(kc == n_kc - 1),
                                skip_group_check=(m > 0))
                            if m == 1 and mode != "peonly":
                                # z a few AV matmuls early: ones[128,128]
                                # writes the denominator row to every PSUM
                                # partition (broadcast free), and the
                                # reciprocal hides under remaining AVs.
                                # zb borrows a score slot: no score allocs
                                # remain in this unit after m==1 fires.
                                zb_ps = stpool.tile([128, QB], F32,
                                                    tag="stps", name="zbps")
                                nc.tensor.matmul(zb_ps[:], (ones[:]),
                                                 (esum[:]),
                                                 start=True, stop=True)
                                nc.vector.reciprocal(zbr[:], zb_ps[:])
                        if mode == "peonly":
                            nc.scalar.copy(ot_sb[:], ot_ps[:])
                        else:
                            nc.vector.tensor_mul(ot_sb[:], ot_ps[:], zbr[:])
                        ot_sbs.append(ot_sb)
                    if mode == "noop":
                        return
                    for c4 in range(QB // 128):
                        row0 = b * seq + q0 + c4 * 128
                        # one [128, H] stage per 128-token chunk: 4 PSUM
                        # evictions (split ACT/DVE) then a single fat store
                        # (4KB DRAM lines)
                        stg = stage_pool.tile([128, H], BF16,
                                              tag="stage", name="stg")
                        last = b == B - 1 and qb == nblocks - 1
                        # oc pairs with hh outer: consecutive matmuls share
                        # the same stationary ot chunk — repeated LDWEIGHTS
                        # of identical weights measure ~76ns cheaper.
                        for ocp in range(H // QB // 2):
                            ops = [oppool.tile([128, QB], F32, tag="opps",
                                               name="opps")
                                   for _ in range(2)]
                            for hh in range(HPC):
                                for j in range(2):
                                    oc = 2 * ocp + j
                                    nc.tensor.matmul(
                                        ops[j][:],
                                        (ot_sbs[hh][:,
                                                    c4 * 128:(c4 + 1) * 128]),
                                        (woT[:, hh * H + oc * QB:
                                               hh * H + (oc + 1) * QB]),
                                        start=(hh == 0),
                                        stop=(hh == HPC - 1))
                            for j in range(2):
                                oc = 2 * ocp + j
                                sl = stg[:, oc * QB:(oc + 1) * QB]
                                if j == 1:
                                    # DVE carries the esum chain; split the
                                    # PSUM evictions evenly with ACT
                                    nc.scalar.copy(sl, ops[j][:])
                                else:
                                    nc.vector.tensor_copy(sl, ops[j][:])
                        if mode == "nostore":
                            continue
                        # stores ride the ACT queue (SP carries x loads);
                        # final block splits across both for the tail drain
                        eng = nc.sync if (last and c4 % 2) else nc.scalar
                        eng.dma_start(out_ap[row0:row0 + 128, :], stg[:])

                amode = ("full" if only in (None, 'attn') else
                         (only or "").removeprefix('attn_'))
                lag = bd["lag"]
                nP = B * nblocks
                if only == 'proj':
                    order = [("P", i) for i in range(nP)]
                elif (only or "").startswith('attn'):
                    order = [("A", i) for i in range(nP)]
                else:
                    # interleave: attn unit j is emitted after proj unit
                    # j+lag, so attention stalls always have dense proj
                    # matmuls available behind them in the engine streams.
                    order = []
                    for i in range(nP):
                        order.append(("P", i))
                        if i >= lag:
                            order.append(("A", i - lag))
                    for j in range(nP - lag, nP):
                        order.append(("A", j))
                last_p = max((k for k, (kd, _) in enumerate(order)
                              if kd == "P"), default=-1)
                for k, (kind, i) in enumerate(order):
                    if kind == "P":
                        proj_block(i // nblocks, i % nblocks)
                    else:
                        attn_block(i // nblocks, i % nblocks, amode,
                                   deep=(k > last_p))

        if reps == 1:
            body()
        else:
            with tc.For_i(0, reps, 1):
                body()

    nc.compile()
    return nc


def shard_inputs(x, Wq, Wk, Wv, Wo, seq=SEQ):
    t = B * seq
    bf = ml_dtypes.bfloat16
    x2t = np.asarray(x, dtype=np.float32).reshape(t, H).T  # [H, t]
    # xb[blk*128+p, k*QB+tt] = x^T[k*128+p, blk*QB+tt] — partition-major
    # blocks so each per-partition DMA line is 16KB contiguous.
    xb = np.ascontiguousarray(
        x2t.reshape(KTH, 128, t // QB, QB).transpose(2, 1, 0, 3)
        .reshape((t // QB) * 128, KTH * QB)).astype(bf)

    def wswz(w):  # [H, DC] -> [128, KTH*DC] partition-major
        return np.ascontiguousarray(
            w.reshape(KTH, 128, DC).transpose(1, 0, 2).reshape(128, KTH * DC)
        ).astype(bf)

    Wq = np.asarray(Wq, dtype=np.float32)
    Wk = np.asarray(Wk, dtype=np.float32)
    Wv = np.asarray(Wv, dtype=np.float32)
    Wo = np.asarray(Wo, dtype=np.float32)
    in_maps = []
    for c in range(N_CORES):
        sl = slice(c * DC, (c + 1) * DC)
        in_maps.append({
            "xb_in": xb,
            "wqb": wswz(np.ascontiguousarray(Wq[sl, :].T)),
            "wkb": wswz(np.ascontiguousarray(Wk[sl, :].T)),
            "wvb": wswz(np.ascontiguousarray(Wv[sl, :].T)),
            "wot": np.ascontiguousarray(Wo[:, sl].T).astype(bf),
        })
    return in_maps


_cache = {}


def kernel(x, Wq, Wk, Wv, Wo):
    if "nc" not in _cache:
        _cache["nc"] = build()
    nc = _cache["nc"]
    in_maps = shard_inputs(x, Wq, Wk, Wv, Wo)
    res = run_bass_kernel_spmd(nc, in_maps, list(range(N_CORES)))
    acc = res.results[0]["out"].astype(np.float32)
    for c in range(1, N_CORES):
        acc = acc + res.results[c]["out"].astype(np.float32)
    return acc.reshape(B, SEQ, H)

